# revision 26
# baseline (speedup 1.0000x reference)
"""Multi-head attention (B=2, S=2048, D=1024, H=16, Dh=64, causal) on 8 TRN2 cores.

Sharding: (batch, head-group) across 8 cores -> core c handles batch c//4 and
heads [4*(c%4), 4*(c%4)+4). Wq/Wk/Wv column-sharded by head group.

Per-core kernel, all matmul operands bf16 (f32 PSUM accumulation):
  - inputs: xtq [4, 128, 8, 512] bf16 (host-transposed x, quarter-major),
    W slices [128, 8, 256] bf16, triangle mask [128, 2, 128] bf16
  - warmup matmuls on zeroed scratch cover the input-DMA latency and the PE
    p-state ramp (full clock needs ~3us of continuous PE busy)
  - projections contract D in 8 chunks of 128; qT/kT produced directly in
    [head*dh, S] bf16 layout; v in [sk, 65] layout with a memset ones column
    (row 64 of the PV accumulator is the softmax normalizer)
  - scoresT tiles [sk=128, sq<=512] = kT.T @ qT per head; causal tiles fully
    above the diagonal are skipped; diagonal-band tiles are trimmed to the
    128*t boundary and their first 128 columns masked by a 0/1 triangle
  - pT = exp(scoresT/8) on the Act engine (bf16 out, no max subtraction:
    scores ~ N(0,1), bf16 range is ample); both heads of a band tile share
    one exp instruction to amortize the Act access-latency penalty
  - ctxT [65, 2, sq] += v_aug.T @ pT accumulated over sk chunks in PSUM;
    query columns are copied out and DMA'd incrementally as each band tile
    finalizes them; the host divides rows 0..63 by the normalizer row 64
  - the attention streams run in sq-tile order [1, 2, 3, 0]; projection
    matmul groups are interleaved into earlier streams as PE filler (the
    exp stream on Act runs ~185ns/group longer than the matching PE work,
    so the in-order PE queue needs independent work to never stall), with
    deadlines so every projection lands before its first consumer
"""
import sys

if "/opt/trn_rl_repo" not in sys.path:
    sys.path.insert(0, "/opt/trn_rl_repo")

import numpy as np
import ml_dtypes

import concourse.bacc as bacc
import concourse.mybir as mybir
import concourse.tile as tile
from concourse.bass_utils import run_bass_kernel_spmd

F32 = mybir.dt.float32
BF16 = mybir.dt.bfloat16
F32R = mybir.dt.float32r

# matmul operand dtype: "bf16" or "f32r" (f32r self-loads weights, no
# separate Ldweights; <256-col matmuls run 4x slower so band trim differs)
DT_MODE = "bf16"

P = 128          # partitions / sk chunk
S = 2048         # sequence length
D = 1024         # model dim
C = 256          # W columns per core (4 heads x 64)
DH = 64          # head dim
NH = 4           # heads per core
SQT = 512        # sq tile (matmul free dim)
NSQ = S // SQT   # 4
NSK = S // P     # 16
ND = D // P      # 8
N_CORES = 8
N_WARM = 34      # warmup matmuls: cover input-DMA latency (~7.3us)

_NC_CACHE = {}


def build_nc(loop_n=1, dt_mode=None):
    dt_mode = dt_mode or DT_MODE
    key = ("nc", loop_n, dt_mode)
    if key in _NC_CACHE:
        return _NC_CACHE[key]
    DT = BF16 if dt_mode == "bf16" else F32R
    MINW = 0 if dt_mode == "bf16" else 2 * P
    nc = bacc.Bacc("TRN2")
    xtq = nc.dram_tensor("xtq", [NSQ, P, ND, SQT], DT, kind="ExternalInput")
    wq = nc.dram_tensor("wq", [P, ND, C], DT, kind="ExternalInput")
    wk = nc.dram_tensor("wk", [P, ND, C], DT, kind="ExternalInput")
    wv = nc.dram_tensor("wv", [P, ND, C], DT, kind="ExternalInput")
    masks = nc.dram_tensor("masks", [P, 2, P], DT, kind="ExternalInput")
    octxT = nc.dram_tensor("octxT", [NSQ, 2, DH + 1, 2, SQT], F32,
                           kind="ExternalOutput")

    import contextlib
    with tile.TileContext(nc) as tc:
        with (tc.For_i(0, loop_n, 1) if loop_n > 1 else contextlib.nullcontext()), \
             tc.tile_pool(name="const", bufs=1) as cp, \
             tc.tile_pool(name="work", bufs=2) as wkp, \
             tc.tile_pool(name="ps", bufs=2, space="PSUM") as ps:
            # ---- persistent SBUF residents ----
            xt = cp.tile([P, ND, S], DT, tag="xt", name="xt")
            wq_sb = cp.tile([P, ND, C], DT, tag="wq", name="wq_sb")
            wk_sb = cp.tile([P, ND, C], DT, tag="wk", name="wk_sb")
            wv_sb = cp.tile([P, ND, C], DT, tag="wv", name="wv_sb")
            mask_sb = cp.tile([P, 2, P], DT, tag="mask", name="mask_sb")
            qT_sb = [cp.tile([P, S], DT, tag=f"qT{i}", name=f"qT{i}")
                     for i in range(2)]
            kT_sb = [cp.tile([P, S], DT, tag=f"kT{i}", name=f"kT{i}")
                     for i in range(2)]
            va = cp.tile([P, NSK, NH, DH + 1], DT, tag="va", name="va")
            warm_sb = cp.tile([P, C], DT, tag="warm", name="warm_sb")

            # ---- scratch init (DVE) ----
            nc.vector.memset(warm_sb[:], 0.0)
            nc.vector.memset(va[:, :, :, DH], 1.0)

            # ---- input DMAs ----
            # sync queue: x quarters (q0 split for earlier start)
            # HBM bandwidth is one shared serial resource: issue transfers
            # in first-use order on the sync queue (prologue: wv+xt q0 for
            # v, wk for kT(0), xt q1 + wq for qT(1))
            nc.scalar.dma_start(wv_sb[:], wv[:])
            nc.sync.dma_start(xt[:, 0:4, 0:SQT], xtq[0, :, 0:4, :])
            nc.sync.dma_start(xt[:, 4:8, 0:SQT], xtq[0, :, 4:8, :])
            nc.gpsimd.dma_start(wk_sb[:], wk[:])
            nc.sync.dma_start(xt[:, 0:4, SQT:2 * SQT], xtq[1, :, 0:4, :])
            nc.scalar.dma_start(wq_sb[:], wq[:])
            nc.sync.dma_start(xt[:, 4:8, SQT:2 * SQT], xtq[1, :, 4:8, :])
            for q in range(2, NSQ):
                nc.sync.dma_start(xt[:, :, q * SQT:(q + 1) * SQT], xtq[q])
            nc.scalar.dma_start(mask_sb[:], masks[:])

            # ---- warmup: keep PE busy + ramping while DMAs land ----
            for _ in range(N_WARM):
                s = ps.tile([P, SQT], F32, tag="B", bufs=2, name="warm")
                nc.tensor.matmul(s[:, 0:C], warm_sb[:, 0:P], warm_sb[:],
                                 start=True, stop=True)

            # ---- projection emit helpers ----
            def proj_v_mms(j, slot, ks):
                for k in ks:
                    nc.tensor.matmul(slot[:, 0:C],
                                     xt[:, k, j * P:(j + 1) * P],
                                     wv_sb[:, k],
                                     start=(k == 0), stop=(k == ND - 1))

            def proj_v_copy(j, slot):
                nc.vector.tensor_copy(
                    va[:, j, :, 0:DH],
                    slot[:, 0:C].rearrange("p (h d) -> p h d", h=NH))

            def bslot(name):
                return ps.tile([P, SQT], F32, tag="B", bufs=2, name=name)

            def proj_qk_mms(w_sb, hp, c, slot, ks):
                sq = slice(c * SQT, (c + 1) * SQT)
                for k in ks:
                    nc.tensor.matmul(slot[:],
                                     w_sb[:, k, hp * P:(hp + 1) * P],
                                     xt[:, k, sq],
                                     start=(k == 0), stop=(k == ND - 1))

            def proj_qk_copy(dst, hp, c, slot):
                sq = slice(c * SQT, (c + 1) * SQT)
                nc.vector.tensor_copy(dst[hp][:, sq], slot[:])

            def proj_qk(w_sb, dst, hp, c):
                slot = bslot("pj")
                proj_qk_mms(w_sb, hp, c, slot, range(ND))
                proj_qk_copy(dst, hp, c, slot)

            # filler units: (fn, stride, deadline) halves (~430/850ns PE)
            def v_filler(j, deadline=None):
                box = {}

                def f1():
                    box["s"] = bslot("pv")
                    proj_v_mms(j, box["s"], range(4))

                def f2():
                    proj_v_mms(j, box["s"], range(4, ND))
                    proj_v_copy(j, box["s"])
                return [(f1, 3, deadline), (f2, 3, deadline)]

            def qk_filler(w_sb, dst, hp, c, stride=5, deadline=None):
                box = {}

                def f1():
                    box["s"] = bslot("pjf")
                    proj_qk_mms(w_sb, hp, c, box["s"], range(4))

                def f2():
                    proj_qk_mms(w_sb, hp, c, box["s"], range(4, ND))
                    proj_qk_copy(dst, hp, c, box["s"])
                return [(f1, stride, deadline), (f2, stride, deadline)]

            def make_attn_groups(c, pscs_ref):
                """Attention groups for sq tile c: list of
                (emit_qk, emit_exp_mask, emit_pv) triples."""
                sq = slice(c * SQT, (c + 1) * SQT)
                jmax = 4 * c + 4
                groups = []
                oc_ref = {}

                def get_pscs(hp):
                    # one [65, 2, SQT] accumulator per (c, hp); bufs=1 so
                    # successive hp groups WAR-serialize on the output copies
                    if pscs_ref[0] is None or pscs_ref[1] != hp:
                        pscs_ref[0] = ps.tile([DH + 1, 2, SQT], F32, tag="A",
                                              bufs=1, name="pscs")
                        pscs_ref[1] = hp
                    return pscs_ref[0]

                def u_group(hp, i, jp):
                    j0 = 2 * jp
                    off = DH * i
                    box = {}

                    def qk():
                        box["pscs"] = get_pscs(hp)
                        pss = ps.tile([P, 2, SQT], F32, tag="S", bufs=2,
                                      name="pss")
                        for u in range(2):
                            nc.tensor.matmul(
                                pss[:, u],
                                kT_sb[hp][off:off + DH,
                                          (j0 + u) * P:(j0 + u + 1) * P],
                                qT_sb[hp][off:off + DH, sq],
                                start=True, stop=True)
                        box["pss"] = pss

                    def ex():
                        pt = wkp.tile([P, 2, SQT], DT, tag="pT", bufs=8,
                                      name="pt")
                        nc.scalar.activation(pt[:], box["pss"][:],
                                             mybir.ActivationFunctionType.Exp,
                                             scale=0.125)
                        box["pt"] = pt

                    def pv():
                        h = 2 * hp + i
                        for u in range(2):
                            nc.tensor.matmul(box["pscs"][:, i],
                                             va[:, j0 + u, h, :],
                                             box["pt"][:, u],
                                             start=(j0 + u == 0), stop=False)

                    return (qk, ex, pv)

                def b_group(hp, t):
                    # both heads (i=0,1) of band tile t in one group
                    j = 4 * c + t
                    lo = min(P * t, SQT - MINW) if MINW else P * t
                    tri = P * t
                    w = SQT - lo
                    seg = slice(P * t, P * t + P)
                    box = {}

                    def qk():
                        box["pscs"] = get_pscs(hp)
                        pss = ps.tile([P, 2, SQT], F32, tag="S", bufs=2,
                                      name="psb")
                        for i in range(2):
                            off = DH * i
                            nc.tensor.matmul(
                                pss[:, i, 0:w],
                                kT_sb[hp][off:off + DH, j * P:(j + 1) * P],
                                qT_sb[hp][off:off + DH,
                                          c * SQT + lo:(c + 1) * SQT],
                                start=True, stop=True)
                        box["pss"] = pss

                    def ex():
                        pt = wkp.tile([P, 2, SQT], DT, tag="pT", bufs=8,
                                      name="ptb")
                        nc.scalar.activation(pt[:, :, lo:SQT],
                                             box["pss"][:, :, 0:w],
                                             mybir.ActivationFunctionType.Exp,
                                             scale=0.125)
                        nc.vector.tensor_mul(pt[:, :, tri:tri + P],
                                             pt[:, :, tri:tri + P],
                                             mask_sb[:])
                        box["pt"] = pt

                    def pv():
                        pscs = box["pscs"]
                        for i in range(2):
                            nc.tensor.matmul(pscs[:, i, lo:SQT],
                                             va[:, j, 2 * hp + i, :],
                                             box["pt"][:, i, lo:SQT],
                                             start=(j == 0),
                                             stop=(j == jmax - 1))
                        # ship the accumulator once the group closes (PSUM
                        # reads mid-accumulation-group are illegal); two
                        # column halves so the first DMA overlaps the
                        # second copy
                        if t == 3:
                            oc = wkp.tile([DH + 1, 2, SQT], F32, tag="oc",
                                          bufs=2, name="oc")
                            half = SQT // 2
                            for u in range(2):
                                cs = slice(u * half, (u + 1) * half)
                                nc.vector.tensor_copy(oc[:, :, cs],
                                                      pscs[:, :, cs])
                                nc.sync.dma_start(octxT[c, hp, :, :, cs],
                                                  oc[:, :, cs])

                    return (qk, ex, pv)

                for hp in range(2):
                    for jp in range(2 * c):
                        for i in range(2):
                            groups.append(u_group(hp, i, jp))
                    for t in range(4):
                        groups.append(b_group(hp, t))
                return groups

            def run_pipeline(groups, fillers):
                # fillers: (fn, stride, deadline); schedule each at cumulative
                # stride positions clamped to its deadline (the last group
                # position before a consumer of its output is emitted),
                # leftovers after the last group
                n = len(groups)
                sched = {}
                pos = -3
                for idx, (fn, stride, deadline) in enumerate(fillers):
                    pos += stride
                    key = max(0, min(pos, n + idx + 1))
                    if deadline is not None:
                        key = min(key, deadline)
                        pos = min(pos, deadline)
                    sched.setdefault(key, []).append(fn)
                for g in range(min(2, n)):
                    groups[g][0]()
                    groups[g][1]()
                for fn in sched.pop(0, []):
                    fn()
                for g in range(n):
                    groups[g][2]()
                    for fn in sched.pop(g + 1, []):
                        fn()
                    if g + 2 < n:
                        groups[g + 2][0]()
                        groups[g + 2][1]()
                for key in sorted(sched):
                    for fn in sched[key]:
                        fn()

            # ---- prologue (stream order is [1, 2, 3, 0]):
            # A(1) needs v(0..3), kT(0), qT(1). All four v k0-3 halves start
            # on the first two xt ko-pair DMAs (B plus idle S slots), k4-7
            # rounds follow DMA arrival.
            vs = {0: bslot("pv0"), 1: bslot("pv0")}
            st = {2: ps.tile([P, 2, SQT], F32, tag="S", bufs=2, name="pvs"),
                  3: ps.tile([P, 2, SQT], F32, tag="S", bufs=2, name="pvs")}
            vslot = {0: vs[0], 1: vs[1],
                     2: st[2][:, 0, :], 3: st[3][:, 0, :]}
            for j in range(4):
                proj_v_mms(j, vslot[j], range(4))
            for j in range(4):
                proj_v_mms(j, vslot[j], range(4, ND))
                proj_v_copy(j, vslot[j])
            for hp in range(2):
                proj_qk(wk_sb, kT_sb, hp, 0)
                proj_qk(wq_sb, qT_sb, hp, 1)

            # ---- per-stream fillers ----
            # deficit: Act runs ~185ns/group longer than PE per attention
            # group; spread projection filler work across each stream,
            # respecting first-consumer deadlines. run_pipeline emits the
            # QK of group g+3 right after the key-(g+1) fillers, so a
            # filler consumed by a QK stage of group R needs key <= R-2;
            # one consumed by a PV stage (the v fillers) needs key <= R-1.
            def v_same(c):
                out = []
                for t in range(4):
                    out.extend(v_filler(4 * c + t, deadline=4 * c + t))
                return out

            filler_map = {
                1: (qk_filler(wk_sb, kT_sb, 0, 1, stride=2, deadline=3)
                    + v_same(1)
                    + qk_filler(wk_sb, kT_sb, 1, 1, stride=3, deadline=11)
                    + qk_filler(wq_sb, qT_sb, 0, 2, stride=2)
                    + qk_filler(wq_sb, qT_sb, 1, 2, stride=2)),
                2: (qk_filler(wk_sb, kT_sb, 0, 2, stride=2, deadline=7)
                    + v_same(2)
                    + qk_filler(wk_sb, kT_sb, 1, 2, stride=4, deadline=19)
                    + qk_filler(wq_sb, qT_sb, 0, 3, stride=4)
                    + qk_filler(wq_sb, qT_sb, 1, 3, stride=4)),
                3: (qk_filler(wk_sb, kT_sb, 0, 3, stride=2, deadline=11)
                    + v_same(3)
                    + qk_filler(wk_sb, kT_sb, 1, 3, stride=4, deadline=27)
                    + qk_filler(wq_sb, qT_sb, 0, 0, stride=5)),
                0: qk_filler(wq_sb, qT_sb, 1, 0, stride=2, deadline=3),
            }

            for c in (1, 2, 3, 0):
                pscs_ref = [None, None]
                groups = make_attn_groups(c, pscs_ref)
                run_pipeline(groups, filler_map[c])
    nc.compile()
    _NC_CACHE[key] = nc
    return nc


def make_in_maps(x, Wq, Wk, Wv, dt_mode=None):
    dt_mode = dt_mode or DT_MODE
    bf = ml_dtypes.bfloat16 if dt_mode == "bf16" else np.float32
    x = np.asarray(x, dtype=np.float32)
    Wq = np.asarray(Wq, dtype=np.float32)
    Wk = np.asarray(Wk, dtype=np.float32)
    Wv = np.asarray(Wv, dtype=np.float32)
    tri = (np.arange(P)[:, None] <= np.arange(P)[None, :])
    masks = np.ascontiguousarray(
        np.broadcast_to(tri[:, None, :], (P, 2, P)).astype(bf))
    in_maps = []
    for core in range(N_CORES):
        b, g = divmod(core, 4)
        cols = slice(C * g, C * (g + 1))
        xtq = np.ascontiguousarray(
            x[b].T.reshape(ND, P, NSQ, SQT).transpose(2, 1, 0, 3).astype(bf))
        in_maps.append({
            "xtq": xtq,
            "wq": np.ascontiguousarray(
                Wq[:, cols].reshape(ND, P, C).transpose(1, 0, 2).astype(bf)),
            "wk": np.ascontiguousarray(
                Wk[:, cols].reshape(ND, P, C).transpose(1, 0, 2).astype(bf)),
            "wv": np.ascontiguousarray(
                Wv[:, cols].reshape(ND, P, C).transpose(1, 0, 2).astype(bf)),
            "masks": masks,
        })
    return in_maps


def assemble_out(results):
    out = np.empty((2, S, D), np.float32)
    for core in range(N_CORES):
        b, g = divmod(core, 4)
        oc = results[core]["octxT"]               # [NSQ, 2, DH+1, 2, SQT]
        ctx = oc[:, :, 0:DH] / oc[:, :, DH:DH + 1]   # [NSQ, 2, DH, 2, SQT]
        # [q, hp, d, i, f] -> [q, f, hp, i, d] -> [S, C]
        out[b, :, C * g:C * (g + 1)] = (
            ctx.transpose(0, 4, 1, 3, 2).reshape(S, C))
    return out


def kernel(x, Wq, Wk, Wv):
    nc = build_nc()
    in_maps = make_in_maps(x, Wq, Wk, Wv)
    res = run_bass_kernel_spmd(nc, in_maps, core_ids=list(range(N_CORES)))
    return assemble_out(res.results)


# revision 29
# speedup vs baseline: 3.8694x; 3.8694x over previous
"""Multi-head attention (B=2, S=2048, D=1024, H=16, Dh=64, causal) on 8 TRN2 cores.

Sharding: (batch, head-group) across 8 cores -> core c handles batch c//4 and
heads [4*(c%4), 4*(c%4)+4). Wq/Wk/Wv column-sharded by head group.

Per-core kernel, all matmul operands bf16 (f32 PSUM accumulation):
  - inputs: xtq [4, 128, 8, 512] bf16 (host-transposed x, quarter-major),
    W slices [128, 8, 256] bf16, triangle mask [128, 2, 128] bf16
  - warmup matmuls on zeroed scratch cover the input-DMA latency and the PE
    p-state ramp (full clock needs ~3us of continuous PE busy)
  - projections contract D in 8 chunks of 128; qT/kT produced directly in
    [head*dh, S] bf16 layout; v in [sk, 65] layout with a memset ones column
    (row 64 of the PV accumulator is the softmax normalizer)
  - scoresT tiles [sk=128, sq<=512] = kT.T @ qT per head; causal tiles fully
    above the diagonal are skipped; diagonal-band tiles are trimmed to the
    128*t boundary and their first 128 columns masked by a 0/1 triangle
  - pT = exp(scoresT/8) on the Act engine (bf16 out, no max subtraction:
    scores ~ N(0,1), bf16 range is ample); both heads of a band tile share
    one exp instruction to amortize the Act access-latency penalty
  - ctxT [65, 2, sq] += v_aug.T @ pT accumulated over sk chunks in PSUM,
    copied out and DMA'd unnormalized in two column halves once the
    accumulation group closes; the host divides rows 0..63 by row 64
  - the attention streams run in sq-tile order [1, 2, 3, 0]; projection
    matmul groups are interleaved into earlier streams as PE filler (the
    exp stream on Act runs ~185ns/group longer than the matching PE work,
    so the in-order PE queue needs independent work to never stall), with
    deadlines so every projection lands before its first consumer
"""
import sys

if "/opt/trn_rl_repo" not in sys.path:
    sys.path.insert(0, "/opt/trn_rl_repo")

import numpy as np
import ml_dtypes

import concourse.bacc as bacc
import concourse.mybir as mybir
import concourse.tile as tile
from concourse.bass_utils import run_bass_kernel_spmd

F32 = mybir.dt.float32
BF16 = mybir.dt.bfloat16
F32R = mybir.dt.float32r

# matmul operand dtype: "bf16" or "f32r" (f32r self-loads weights, no
# separate Ldweights; <256-col matmuls run 4x slower so band trim differs)
DT_MODE = "bf16"

P = 128          # partitions / sk chunk
S = 2048         # sequence length
D = 1024         # model dim
C = 256          # W columns per core (4 heads x 64)
DH = 64          # head dim
NH = 4           # heads per core
SQT = 512        # sq tile (matmul free dim)
NSQ = S // SQT   # 4
NSK = S // P     # 16
ND = D // P      # 8
N_CORES = 8
N_WARM = 23      # warmup matmuls: cover input-DMA latency

_NC_CACHE = {}


def build_nc(loop_n=1, dt_mode=None):
    dt_mode = dt_mode or DT_MODE
    key = ("nc", loop_n, dt_mode)
    if key in _NC_CACHE:
        return _NC_CACHE[key]
    DT = BF16 if dt_mode == "bf16" else F32R
    MINW = 0 if dt_mode == "bf16" else 2 * P
    nc = bacc.Bacc("TRN2")
    xtq = nc.dram_tensor("xtq", [NSQ, P, ND, SQT], DT, kind="ExternalInput")
    wq = nc.dram_tensor("wq", [P, ND, C], DT, kind="ExternalInput")
    wk = nc.dram_tensor("wk", [P, ND, C], DT, kind="ExternalInput")
    wv = nc.dram_tensor("wv", [P, ND, C], DT, kind="ExternalInput")
    masks = nc.dram_tensor("masks", [P, 2, P], DT, kind="ExternalInput")
    octxT = nc.dram_tensor("octxT", [NSQ, 2, DH + 1, 2, SQT], F32,
                           kind="ExternalOutput")

    import contextlib
    with tile.TileContext(nc) as tc:
        with (tc.For_i(0, loop_n, 1) if loop_n > 1 else contextlib.nullcontext()), \
             tc.tile_pool(name="const", bufs=1) as cp, \
             tc.tile_pool(name="work", bufs=2) as wkp, \
             tc.tile_pool(name="ps", bufs=2, space="PSUM") as ps:
            # ---- persistent SBUF residents ----
            xt = cp.tile([P, ND, S], DT, tag="xt", name="xt")
            wq_sb = cp.tile([P, ND, C], DT, tag="wq", name="wq_sb")
            wk_sb = cp.tile([P, ND, C], DT, tag="wk", name="wk_sb")
            wv_sb = cp.tile([P, ND, C], DT, tag="wv", name="wv_sb")
            mask_sb = cp.tile([P, 2, P], DT, tag="mask", name="mask_sb")
            qT_sb = [cp.tile([P, S], DT, tag=f"qT{i}", name=f"qT{i}")
                     for i in range(2)]
            kT_sb = [cp.tile([P, S], DT, tag=f"kT{i}", name=f"kT{i}")
                     for i in range(2)]
            va = cp.tile([P, NSK, NH, DH + 1], DT, tag="va", name="va")
            warm_sb = cp.tile([P, C], DT, tag="warm", name="warm_sb")

            # ---- scratch init (DVE) ----
            nc.vector.memset(warm_sb[:], 0.0)
            nc.vector.memset(va[:, :, :, DH], 1.0)

            # ---- input DMAs ----
            # sync queue: x quarters (q0 split for earlier start)
            # HBM bandwidth is one shared serial resource: issue transfers
            # in first-use order on the sync queue (prologue: wv+xt q0 for
            # v, wk for kT(0), xt q1 + wq for qT(1))
            nc.sync.dma_start(wv_sb[:], wv[:])
            nc.sync.dma_start(xt[:, 0:4, 0:SQT], xtq[0, :, 0:4, :])
            nc.sync.dma_start(xt[:, 4:8, 0:SQT], xtq[0, :, 4:8, :])
            nc.sync.dma_start(wk_sb[:], wk[:])
            nc.sync.dma_start(wq_sb[:], wq[:])
            nc.sync.dma_start(xt[:, 0:4, SQT:2 * SQT], xtq[1, :, 0:4, :])
            nc.sync.dma_start(xt[:, 4:8, SQT:2 * SQT], xtq[1, :, 4:8, :])
            for q in range(2, NSQ):
                nc.sync.dma_start(xt[:, :, q * SQT:(q + 1) * SQT], xtq[q])
            nc.sync.dma_start(mask_sb[:], masks[:])

            # ---- warmup: keep PE busy + ramping while DMAs land ----
            for _ in range(N_WARM):
                s = ps.tile([P, SQT], F32, tag="B", bufs=2, name="warm")
                nc.tensor.matmul(s[:, 0:C], warm_sb[:, 0:P], warm_sb[:],
                                 start=True, stop=True)

            # ---- projection emit helpers ----
            def proj_v_mms(j, slot, ks):
                for k in ks:
                    nc.tensor.matmul(slot[:, 0:C],
                                     xt[:, k, j * P:(j + 1) * P],
                                     wv_sb[:, k],
                                     start=(k == 0), stop=(k == ND - 1))

            def proj_v_copy(j, slot):
                nc.vector.tensor_copy(
                    va[:, j, :, 0:DH],
                    slot[:, 0:C].rearrange("p (h d) -> p h d", h=NH))

            def bslot(name):
                return ps.tile([P, SQT], F32, tag="B", bufs=2, name=name)

            def proj_qk_mms(w_sb, hp, c, slot, ks):
                sq = slice(c * SQT, (c + 1) * SQT)
                for k in ks:
                    nc.tensor.matmul(slot[:],
                                     w_sb[:, k, hp * P:(hp + 1) * P],
                                     xt[:, k, sq],
                                     start=(k == 0), stop=(k == ND - 1))

            def proj_qk_copy(dst, hp, c, slot):
                sq = slice(c * SQT, (c + 1) * SQT)
                nc.vector.tensor_copy(dst[hp][:, sq], slot[:])

            def proj_qk(w_sb, dst, hp, c):
                slot = bslot("pj")
                proj_qk_mms(w_sb, hp, c, slot, range(ND))
                proj_qk_copy(dst, hp, c, slot)

            # filler units: (fn, stride, deadline) halves (~430/850ns PE)
            def v_filler(j, deadline=None):
                box = {}

                def f1():
                    box["s"] = bslot("pv")
                    proj_v_mms(j, box["s"], range(4))

                def f2():
                    proj_v_mms(j, box["s"], range(4, ND))
                    proj_v_copy(j, box["s"])
                return [(f1, 3, deadline), (f2, 3, deadline)]

            def qk_filler(w_sb, dst, hp, c, stride=5, deadline=None):
                box = {}

                def f1():
                    box["s"] = bslot("pjf")
                    proj_qk_mms(w_sb, hp, c, box["s"], range(4))

                def f2():
                    proj_qk_mms(w_sb, hp, c, box["s"], range(4, ND))
                    proj_qk_copy(dst, hp, c, box["s"])
                return [(f1, stride, deadline), (f2, stride, deadline)]

            def make_attn_groups(c, pscs_ref):
                """Attention groups for sq tile c: list of
                (emit_qk, emit_exp_mask, emit_pv) triples."""
                sq = slice(c * SQT, (c + 1) * SQT)
                jmax = 4 * c + 4
                groups = []
                oc_ref = {}

                def get_pscs(hp):
                    # one [65, 2, SQT] accumulator per (c, hp); bufs=1 so
                    # successive hp groups WAR-serialize on the output copies
                    if pscs_ref[0] is None or pscs_ref[1] != hp:
                        pscs_ref[0] = ps.tile([DH + 1, 2, SQT], F32, tag="A",
                                              bufs=1, name="pscs")
                        pscs_ref[1] = hp
                    return pscs_ref[0]

                def u_group(hp, i, jp):
                    j0 = 2 * jp
                    off = DH * i
                    box = {}

                    def qk():
                        box["pscs"] = get_pscs(hp)
                        pss = ps.tile([P, 2, SQT], F32, tag="S", bufs=2,
                                      name="pss")
                        for u in range(2):
                            nc.tensor.matmul(
                                pss[:, u],
                                kT_sb[hp][off:off + DH,
                                          (j0 + u) * P:(j0 + u + 1) * P],
                                qT_sb[hp][off:off + DH, sq],
                                start=True, stop=True)
                        box["pss"] = pss

                    def ex():
                        pt = wkp.tile([P, 2, SQT], DT, tag="pT", bufs=8,
                                      name="pt")
                        nc.scalar.activation(pt[:], box["pss"][:],
                                             mybir.ActivationFunctionType.Exp,
                                             scale=0.125)
                        box["pt"] = pt

                    def pv():
                        h = 2 * hp + i
                        for u in range(2):
                            nc.tensor.matmul(box["pscs"][:, i],
                                             va[:, j0 + u, h, :],
                                             box["pt"][:, u],
                                             start=(j0 + u == 0), stop=False)

                    return (qk, ex, pv)

                def b_group(hp, t):
                    # both heads (i=0,1) of band tile t in one group
                    j = 4 * c + t
                    lo = min(P * t, SQT - MINW) if MINW else P * t
                    tri = P * t
                    w = SQT - lo
                    seg = slice(P * t, P * t + P)
                    box = {}

                    def qk():
                        box["pscs"] = get_pscs(hp)
                        pss = ps.tile([P, 2, SQT], F32, tag="S", bufs=2,
                                      name="psb")
                        for i in range(2):
                            off = DH * i
                            nc.tensor.matmul(
                                pss[:, i, 0:w],
                                kT_sb[hp][off:off + DH, j * P:(j + 1) * P],
                                qT_sb[hp][off:off + DH,
                                          c * SQT + lo:(c + 1) * SQT],
                                start=True, stop=True)
                        box["pss"] = pss

                    def ex():
                        pt = wkp.tile([P, 2, SQT], DT, tag="pT", bufs=8,
                                      name="ptb")
                        nc.scalar.activation(pt[:, :, lo:SQT],
                                             box["pss"][:, :, 0:w],
                                             mybir.ActivationFunctionType.Exp,
                                             scale=0.125)
                        nc.vector.tensor_mul(pt[:, :, tri:tri + P],
                                             pt[:, :, tri:tri + P],
                                             mask_sb[:])
                        box["pt"] = pt

                    def pv():
                        pscs = box["pscs"]
                        for i in range(2):
                            nc.tensor.matmul(pscs[:, i, lo:SQT],
                                             va[:, j, 2 * hp + i, :],
                                             box["pt"][:, i, lo:SQT],
                                             start=(j == 0),
                                             stop=(j == jmax - 1))
                        # ship the accumulator once the group closes (PSUM
                        # reads mid-accumulation-group are illegal); two
                        # column halves so the first DMA overlaps the
                        # second copy
                        if t == 3:
                            oc = wkp.tile([DH + 1, 2, SQT], F32, tag="oc",
                                          bufs=2, name="oc")
                            half = SQT // 2
                            for u in range(2):
                                cs = slice(u * half, (u + 1) * half)
                                nc.vector.tensor_copy(oc[:, :, cs],
                                                      pscs[:, :, cs])
                                nc.sync.dma_start(octxT[c, hp, :, :, cs],
                                                  oc[:, :, cs])

                    return (qk, ex, pv)

                for hp in range(2):
                    for jp in range(2 * c):
                        for i in range(2):
                            groups.append(u_group(hp, i, jp))
                    for t in range(4):
                        groups.append(b_group(hp, t))
                return groups

            def run_pipeline(groups, fillers):
                # fillers: (fn, stride, deadline); schedule each at cumulative
                # stride positions clamped to its deadline (the last group
                # position before a consumer of its output is emitted),
                # leftovers after the last group
                n = len(groups)
                sched = {}
                pos = -3
                for idx, (fn, stride, deadline) in enumerate(fillers):
                    pos += stride
                    key = max(0, min(pos, n + idx + 1))
                    if deadline is not None:
                        key = min(key, deadline)
                        pos = min(pos, deadline)
                    sched.setdefault(key, []).append(fn)
                for g in range(min(2, n)):
                    groups[g][0]()
                    groups[g][1]()
                for fn in sched.pop(0, []):
                    fn()
                for g in range(n):
                    groups[g][2]()
                    for fn in sched.pop(g + 1, []):
                        fn()
                    if g + 2 < n:
                        groups[g + 2][0]()
                        groups[g + 2][1]()
                for key in sorted(sched):
                    for fn in sched[key]:
                        fn()

            # ---- prologue (stream order is [1, 2, 3, 0]):
            # A(1) needs v(0..3), kT(0), qT(1). All four v k0-3 halves start
            # on the first two xt ko-pair DMAs (B plus idle S slots), k4-7
            # rounds follow DMA arrival.
            vs = {0: bslot("pv0"), 1: bslot("pv0")}
            st = {2: ps.tile([P, 2, SQT], F32, tag="S", bufs=2, name="pvs"),
                  3: ps.tile([P, 2, SQT], F32, tag="S", bufs=2, name="pvs")}
            vslot = {0: vs[0], 1: vs[1],
                     2: st[2][:, 0, :], 3: st[3][:, 0, :]}
            for j in range(4):
                proj_v_mms(j, vslot[j], range(4))
            for j in range(4):
                proj_v_mms(j, vslot[j], range(4, ND))
                proj_v_copy(j, vslot[j])
            for hp in range(2):
                proj_qk(wk_sb, kT_sb, hp, 0)
                proj_qk(wq_sb, qT_sb, hp, 1)

            # ---- per-stream fillers ----
            # deficit: Act runs ~185ns/group longer than PE per attention
            # group; spread projection filler work across each stream,
            # respecting first-consumer deadlines. run_pipeline emits the
            # QK of group g+3 right after the key-(g+1) fillers, so a
            # filler consumed by a QK stage of group R needs key <= R-2;
            # one consumed by a PV stage (the v fillers) needs key <= R-1.
            def v_same(c):
                out = []
                for t in range(4):
                    out.extend(v_filler(4 * c + t, deadline=4 * c + t))
                return out

            filler_map = {
                1: (qk_filler(wk_sb, kT_sb, 0, 1, stride=2, deadline=3)
                    + v_same(1)
                    + qk_filler(wk_sb, kT_sb, 1, 1, stride=3, deadline=11)
                    + qk_filler(wq_sb, qT_sb, 0, 2, stride=2)
                    + qk_filler(wq_sb, qT_sb, 1, 2, stride=2)),
                2: (qk_filler(wk_sb, kT_sb, 0, 2, stride=2, deadline=7)
                    + v_same(2)
                    + qk_filler(wk_sb, kT_sb, 1, 2, stride=4, deadline=19)
                    + qk_filler(wq_sb, qT_sb, 0, 3, stride=4)
                    + qk_filler(wq_sb, qT_sb, 1, 3, stride=4)),
                3: (qk_filler(wk_sb, kT_sb, 0, 3, stride=2, deadline=11)
                    + v_same(3)
                    + qk_filler(wk_sb, kT_sb, 1, 3, stride=4, deadline=27)
                    + qk_filler(wq_sb, qT_sb, 0, 0, stride=5)),
                0: qk_filler(wq_sb, qT_sb, 1, 0, stride=2, deadline=3),
            }

            for c in (1, 2, 3, 0):
                pscs_ref = [None, None]
                groups = make_attn_groups(c, pscs_ref)
                run_pipeline(groups, filler_map[c])
    nc.compile()
    _NC_CACHE[key] = nc
    return nc


def make_in_maps(x, Wq, Wk, Wv, dt_mode=None):
    dt_mode = dt_mode or DT_MODE
    bf = ml_dtypes.bfloat16 if dt_mode == "bf16" else np.float32
    x = np.asarray(x, dtype=np.float32)
    Wq = np.asarray(Wq, dtype=np.float32)
    Wk = np.asarray(Wk, dtype=np.float32)
    Wv = np.asarray(Wv, dtype=np.float32)
    tri = (np.arange(P)[:, None] <= np.arange(P)[None, :])
    masks = np.ascontiguousarray(
        np.broadcast_to(tri[:, None, :], (P, 2, P)).astype(bf))
    in_maps = []
    for core in range(N_CORES):
        b, g = divmod(core, 4)
        cols = slice(C * g, C * (g + 1))
        xtq = np.ascontiguousarray(
            x[b].T.reshape(ND, P, NSQ, SQT).transpose(2, 1, 0, 3).astype(bf))
        in_maps.append({
            "xtq": xtq,
            "wq": np.ascontiguousarray(
                Wq[:, cols].reshape(ND, P, C).transpose(1, 0, 2).astype(bf)),
            "wk": np.ascontiguousarray(
                Wk[:, cols].reshape(ND, P, C).transpose(1, 0, 2).astype(bf)),
            "wv": np.ascontiguousarray(
                Wv[:, cols].reshape(ND, P, C).transpose(1, 0, 2).astype(bf)),
            "masks": masks,
        })
    return in_maps


def assemble_out(results):
    out = np.empty((2, S, D), np.float32)
    for core in range(N_CORES):
        b, g = divmod(core, 4)
        oc = results[core]["octxT"]               # [NSQ, 2, DH+1, 2, SQT]
        ctx = oc[:, :, 0:DH] / oc[:, :, DH:DH + 1]   # [NSQ, 2, DH, 2, SQT]
        # [q, hp, d, i, f] -> [q, f, hp, i, d] -> [S, C]
        out[b, :, C * g:C * (g + 1)] = (
            ctx.transpose(0, 4, 1, 3, 2).reshape(S, C))
    return out


def kernel(x, Wq, Wk, Wv):
    nc = build_nc()
    in_maps = make_in_maps(x, Wq, Wk, Wv)
    res = run_bass_kernel_spmd(nc, in_maps, core_ids=list(range(N_CORES)))
    return assemble_out(res.results)
